# revision 26
# baseline (speedup 1.0000x reference)
"""Local sliding-window attention (B=2, T=2048, D=1024, H=16, window=128)
as a Trainium2 Bass/Tile kernel on 8 NeuronCores.

Sharding: sequence-parallel. Each core owns 512 consecutive tokens of one
batch (4 chunks x 2 batches = 8 cores) plus a 64-token halo of following
tokens (the mask lets query i attend keys [i, i+64]). No collectives.

v3 layout (v2 + startup/PE-work cuts):
  xT        [128, 8, 576] bf16 feature-major input, ONE tile filled by three
            token-range DMAs (0:256 / 256:512 / 512:576) so early projections
            start before the whole tensor lands
  v proj    token-major v_sb[tt] [128, 16, 65] bf16, col 64 = 1.0 (the
            ones column makes the PV matmul also emit softmax denominators)
  q proj    head-pair zero-padded qp_sb[hp] [128, 2, 512] bf16: plane 0 has
            the even head's features on partitions 0:64 (rest zero), plane 1
            the odd head's on 64:128. Full 128-partition contraction keeps
            every S^T matmul in one PE row-group (mixed row-groups hang).
            Bias applied on VectorE (tensor_scalar add, per-partition).
  k proj    feature-major kt_sb[hp] [128, 576] bf16, chunks 256+320
  attention per (head-pair, 128-query block): S^T = k.T q as ONE matmul per
            key tile (3D moving AP covers both head planes), exp on ScalarE
            (no additive mask), 0/1 band mask applied post-exp on VectorE,
            PV matmul emits unnormalized output + denominator row,
            1/den = exp(-ln(den)) on ScalarE, PE broadcast, normalize muls
  out proj  stationary attn tiles (feature-major) x moving w_outT, biases
            via rank-1 ones-row matmuls, per-hf output DMA

Host-side prep (numpy): transposes, bf16 casts, q-scale folding, 0/1 bands.
"""

import numpy as np
import ml_dtypes

BF = ml_dtypes.bfloat16

N_CORES = 8
B, T, D = 2, 2048, 1024
H, HD = 16, 64
W2 = 64            # window_size // 2 (look-ahead span)
TC = T // 4        # 512 own tokens per core
TH = TC + W2       # 576 with halo
NQB = TC // 128    # 4 query blocks per head
ND = D // 128      # 8 contraction tiles

_CACHED = {}


def _patch_framework(bass, mybir, tile):
    """Work around this walrus build's 1-sync-wait-per-instruction limit."""
    from concourse.vector_clock import ScopedClock

    if getattr(tile.TileContext, "_swa_patched", False):
        return

    def _drain_and_barrier(self, tick_clock, wait_clock):
        nc = self.nc
        drain_inst = nc.sync.drain()
        wait_clock.add_sem_waits(
            drain_inst.ins, ScopedClock({None: tick_clock.global_clock})
        )
        si = drain_inst.ins.sync_info
        waits = list(si.on_wait)
        if len(waits) > 1:
            si.on_wait = [waits[0]]
            for w in waits[1:]:
                extra = nc.sync.drain()
                extra.ins.sync_info = type(si)(on_wait=[w], on_update=[])
        nc.all_engine_barrier()
        assert self.sems is not None
        popped = nc._tile_sem_poison_stack.pop()
        assert popped is self._sem_poison
        nc.clear_and_free_semaphores(list(self.sems.allocated().values()))
        nc.all_engine_barrier()

    tile.TileContext._drain_and_barrier = _drain_and_barrier
    tile.TileContext._swa_patched = True


def _split_multiwaits(nc, mybir):
    """Hoist excess sync waits onto same-engine NOPs before the instruction."""
    n = 0
    for fn in nc.m.functions:
        for bb in fn.blocks:
            insts = bb.instructions
            new_list = []
            changed = False
            for inst in insts:
                si = inst.sync_info
                nw = len(si.on_wait) if si is not None and si.on_wait else 0
                if nw > 1:
                    waits = list(si.on_wait)
                    for j, w in enumerate(waits[:-1]):
                        nop = mybir.InstNoOp(
                            name=f"{inst.name}-wsplit{j}", ins=[], outs=[]
                        )
                        nop.engine = inst.engine
                        nop.sync_info = mybir.SyncInfo(on_wait=[w], on_update=[])
                        new_list.append(nop)
                        n += 1
                    si.on_wait = waits[-1:]
                    changed = True
                new_list.append(inst)
            if changed:
                insts.clear()
                insts.extend(new_list)
    return n


def _build_nc():
    import concourse.bass as bass
    import concourse.mybir as mybir
    import concourse.tile as tile

    _patch_framework(bass, mybir, tile)

    F32 = mybir.dt.float32
    BF16 = mybir.dt.bfloat16
    AF = mybir.ActivationFunctionType

    nc = bass.Bass("TRN2")

    xT_d = nc.dram_tensor("xT", [128, ND, TH], BF16, kind="ExternalInput")
    wqk_d = nc.dram_tensor("w_qk", [16, 128, ND, 128], BF16, kind="ExternalInput")
    wv_d = nc.dram_tensor("w_v", [2, 128, ND, 512], BF16, kind="ExternalInput")
    bqk_d = nc.dram_tensor("b_qk", [128, 16], F32, kind="ExternalInput")
    wo_d = nc.dram_tensor("w_o", [128, ND, D], BF16, kind="ExternalInput")
    # blob1: band1 [128,256] at cols 0:256; band2 [64,2,256] rows 0:64 cols 256:768
    blob1_d = nc.dram_tensor("blob1", [128, 768], BF16, kind="ExternalInput")
    # blob2 (partition 0): ones 0:128 | b_v 128:1152 | b_out 1152:2176
    blob2_d = nc.dram_tensor("blob2", [1, 2176], BF16, kind="ExternalInput")
    out_d = nc.dram_tensor("out", [TC, D], F32, kind="ExternalOutput")

    with tile.TileContext(nc) as tc:
        with (
            tc.tile_pool(name="persist", bufs=1) as persist,
            tc.tile_pool(name="consts", bufs=1) as consts,
            tc.tile_pool(name="wcol", bufs=4) as wcol_pool,
            tc.tile_pool(name="wv", bufs=2) as wv_pool,
            tc.tile_pool(name="wo", bufs=1) as wo_pool,
            tc.tile_pool(name="psmm", bufs=3, space="PSUM") as mm_pool,
            tc.tile_pool(name="psst", bufs=2, space="PSUM") as st_pool,
            tc.tile_pool(name="psop", bufs=3, space="PSUM") as op_pool,
            tc.tile_pool(name="pp", bufs=3) as p_pool,
            tc.tile_pool(name="rdp", bufs=3) as rd_pool,
        ):
            # ---- persistent SBUF ----
            xT = persist.tile([128, ND, TH], BF16, tag="xT", name="xT")
            # split by token range so early consumers start sooner
            nc.sync.dma_start(xT[:, :, 0:256], xT_d[:, :, 0:256])

            _wcols = {}

            def load_wcol(ft):
                wcol = wcol_pool.tile([128, ND, 128], BF16, tag="wcol")
                nc.sync.dma_start(wcol[:], wqk_d[ft])
                _wcols[ft] = wcol

            load_wcol(8)
            load_wcol(0)

            bqk = consts.tile([128, 16], F32, tag="bqk")
            nc.sync.dma_start(bqk[:], bqk_d[:])
            nc.sync.dma_start(xT[:, :, 256:512], xT_d[:, :, 256:512])
            nc.sync.dma_start(xT[:, :, 512:576], xT_d[:, :, 512:576])

            wvs = [
                wv_pool.tile([128, ND, 512], BF16, tag="wv", name=f"wv{i}")
                for i in range(2)
            ]
            nc.sync.dma_start(wvs[0][:], wv_d[0])  # wv1 DMA'd later

            blob2 = consts.tile([1, 2176], BF16, tag="blob2")
            nc.sync.dma_start(blob2[:], blob2_d[:])
            blob1 = consts.tile([128, 768], BF16, tag="blob1")
            nc.sync.dma_start(blob1[:], blob1_d[:])
            load_wcol(9)
            load_wcol(1)

            ones = blob2[0:1, 0:128]
            band1 = blob1[:, 0:256]

            def band2(mi):
                return blob1[0:64, 256 + 256 * mi : 512 + 256 * mi]

            def bv(hf):
                return blob2[0:1, 128 + 512 * hf : 640 + 512 * hf]

            def bo(hf):
                return blob2[0:1, 1152 + 512 * hf : 1664 + 512 * hf]

            qp_sb = [
                persist.tile([128, 2, TC], BF16, tag=f"qp{hp}", name=f"qp{hp}")
                for hp in range(8)
            ]
            kt_sb = [
                persist.tile([128, TH], BF16, tag=f"kt{hp}", name=f"kt{hp}")
                for hp in range(8)
            ]
            v_sb = [
                persist.tile([128, H, HD + 1], BF16, tag=f"v{tt}", name=f"v{tt}")
                for tt in range(5)
            ]
            attn_sb = [
                persist.tile([128, TC], BF16, tag=f"at{pt}", name=f"at{pt}")
                for pt in range(8)
            ]
            out_sb = [
                persist.tile([128, D], F32, tag=f"o{tt}", name=f"o{tt}")
                for tt in range(4)
            ]

            # one-time zero/one fills on the (otherwise idle) GpSimd engine.
            # warm tile first: the HAM warmup matmuls depend on it and must
            # run during the initial DMA wait.
            warm = persist.tile([128, 128], BF16, tag="warm", name="warm")
            nc.gpsimd.memset(warm[0:1, 0:1], 0.0)
            # v ones columns next (needed by the first PV matmuls): they make
            # PV emit softmax denominators in row HD.
            for tt in range(5):
                nc.gpsimd.memset(v_sb[tt][:, :, HD : HD + 1], 1.0)
            for hp in range(8):
                nc.gpsimd.memset(qp_sb[hp][64:128, 0, :], 0.0)
                nc.gpsimd.memset(qp_sb[hp][0:64, 1, :], 0.0)

            # ---- q/k projection, emittable in dt-halves for finer filling ----
            _pcs = {}

            def proj_part(ft, ci, half):
                # q (ft<8): one 512-token chunk (ci==0 only); k: 256+320
                if ft < 8:
                    c0, w = 0, 512
                else:
                    c0, w = (0, 256) if ci == 0 else (256, 320)
                wcol = _wcols[ft]
                if half == 0:
                    ps = mm_pool.tile([128, 512], F32, tag="psmm")
                    _pcs[(ft, ci)] = ps
                    dts = range(0, 4)
                else:
                    ps = _pcs.pop((ft, ci))
                    dts = range(4, ND)
                for dt in dts:
                    nc.tensor.matmul(
                        ps[:, 0:w],
                        wcol[:, dt, :],
                        xT[:, dt, c0 : c0 + w],
                        start=(dt == 0),
                        stop=(dt == ND - 1),
                    )
                if half == 0:
                    return
                if ft < 8:
                    nc.vector.tensor_scalar_add(
                        qp_sb[ft][0:64, 0, c0 : c0 + w],
                        ps[0:64, 0:w],
                        bqk[0:64, ft : ft + 1],
                    )
                    nc.vector.tensor_scalar_add(
                        qp_sb[ft][64:128, 1, c0 : c0 + w],
                        ps[64:128, 0:w],
                        bqk[64:128, ft : ft + 1],
                    )
                else:
                    nc.vector.tensor_scalar_add(
                        kt_sb[ft - 8][:, c0 : c0 + w],
                        ps[:, 0:w],
                        bqk[:, ft : ft + 1],
                    )

            def proj_chunk(ft, ci):
                proj_part(ft, ci, 0)
                proj_part(ft, ci, 1)

            # ---- v projection for one (feature-half, token-tile) ----
            def vproj(hf, tt):
                tsz = 128 if tt < 4 else 64
                ps = mm_pool.tile([128, 512], F32, tag="psmm")
                for dt in range(ND):
                    nc.tensor.matmul(
                        ps[0:tsz, :],
                        xT[:, dt, 128 * tt : 128 * tt + tsz],
                        wvs[hf][:, dt, :],
                        start=(dt == 0),
                        stop=False,
                    )
                nc.tensor.matmul(
                    ps[0:tsz, :],
                    blob2[0:1, 0:tsz],
                    bv(hf),
                    start=False,
                    stop=True,
                )
                nc.vector.tensor_copy(
                    v_sb[tt][0:tsz, 8 * hf : 8 * hf + 8, 0:HD],
                    ps[0:tsz, :].rearrange("p (h f) -> p h f", h=8),
                )

            # ---- attention, software-pipelined in three stages ----
            # fe: S matmuls + exp + band mask   (unit state in _ust)
            # be1 (next slot): PV matmuls + ln/exp reciprocal
            # be2 (slot after): PE broadcast + normalize muls
            _ust = {}

            def attn_fe(u):
                hp, qb = u
                q0 = 128 * qb
                mi = 1 if qb == NQB - 1 else 0
                q2 = qp_sb[hp][:, 0:2, q0 : q0 + 128]  # both head planes
                kt = kt_sb[hp]
                st = st_pool.tile([128, 512], F32, tag="st")
                # S^T tiles: keys on partitions, (plane, query) on free axis.
                nc.tensor.matmul(
                    st[0:128, 0:256], kt[:, q0 : q0 + 128], q2,
                    start=True, stop=True,
                )
                nc.tensor.matmul(
                    st[0:64, 256:512], kt[:, q0 + 128 : q0 + 192], q2,
                    start=True, stop=True,
                )
                par = p_pool.tile([128, 256], BF16, tag="par")
                pbr = p_pool.tile([64, 256], BF16, tag="pbr")
                nc.scalar.activation(par[:], st[0:128, 0:256], AF.Exp)
                nc.scalar.activation(pbr[:], st[0:64, 256:512], AF.Exp)
                # 0/1 band masks applied post-exp (cheaper than mask matmuls)
                pa = p_pool.tile([128, 256], BF16, tag="pa")
                pb = p_pool.tile([64, 256], BF16, tag="pb")
                nc.vector.tensor_mul(pa[:], par[:], band1)
                nc.vector.tensor_mul(pb[:], pbr[:], band2(mi))
                _ust[u] = [pa, pb]

            def attn_be1(u):
                hp, qb = u
                pa, pb = _ust[u]
                op = op_pool.tile([128, 512], F32, tag="op")
                vE1 = v_sb[qb][:, 2 * hp, :]
                vO1 = v_sb[qb][:, 2 * hp + 1, :]
                vE2 = v_sb[qb + 1][0:64, 2 * hp, :]
                vO2 = v_sb[qb + 1][0:64, 2 * hp + 1, :]
                # pa-consumers first (pb's exp+mask lands later); one
                # accumulation group: start=True clears the WHOLE bank's
                # has_written bits, so only the first matmul may set it
                nc.tensor.matmul(
                    op[0 : HD + 1, 0:128], vE1, pa[:, 0:128],
                    start=True, stop=False,
                )
                nc.tensor.matmul(
                    op[0 : HD + 1, 128:256], vO1, pa[:, 128:256],
                    start=False, stop=False, skip_group_check=True,
                )
                nc.tensor.matmul(
                    op[0 : HD + 1, 0:128], vE2, pb[:, 0:128],
                    start=False, stop=False, skip_group_check=True,
                )
                nc.tensor.matmul(
                    op[0 : HD + 1, 128:256], vO2, pb[:, 128:256],
                    start=False, stop=True, skip_group_check=True,
                )
                # 1/den = exp(-ln(den)) on ScalarE
                rd = rd_pool.tile([1, 256], F32, tag="rd")
                nc.scalar.activation(rd[:], op[HD : HD + 1, 0:256], AF.Ln)
                rdb = rd_pool.tile([1, 256], BF16, tag="rdb")
                nc.scalar.activation(rdb[:], rd[:], AF.Exp, scale=-1.0)
                _ust[u] = [op, rdb]

            def attn_be2(u):
                hp, qb = u
                q0 = 128 * qb
                op, rdb = _ust.pop(u)
                nc.tensor.matmul(
                    op[0:64, 256:512],
                    blob2[0:1, 0:64],
                    rdb[:],
                    start=True, stop=True,
                )
                bc = p_pool.tile([64, 256], F32, tag="bc")
                nc.vector.tensor_copy(bc[:], op[0:64, 256:512])
                nc.vector.tensor_mul(
                    attn_sb[hp][0:64, q0 : q0 + 128],
                    op[0:HD, 0:128],
                    bc[:, 0:128],
                )
                nc.vector.tensor_mul(
                    attn_sb[hp][64:128, q0 : q0 + 128],
                    op[0:HD, 128:256],
                    bc[:, 128:256],
                )

            # ---- out projection, split so dt 0..6 (which only need head
            # pairs 0..6) can run as filler before the last attentions ----
            _ops = {}

            def op_head(tt, hf):
                ps = mm_pool.tile([128, 512], F32, tag="psmm")
                _ops[(tt, hf)] = ps
                for dt in range(7):
                    nc.tensor.matmul(
                        ps[:],
                        attn_sb[dt][:, 128 * tt : 128 * tt + 128],
                        _wo[:, dt, 512 * hf : 512 * hf + 512],
                        start=(dt == 0),
                        stop=False,
                    )

            def op_tail(tt, hf, split=False):
                ps = _ops.pop((tt, hf))
                nc.tensor.matmul(
                    ps[:],
                    attn_sb[7][:, 128 * tt : 128 * tt + 128],
                    _wo[:, 7, 512 * hf : 512 * hf + 512],
                    start=False,
                    stop=False,
                )
                nc.tensor.matmul(
                    ps[:], blob2[0:1, 0:128], bo(hf), start=False, stop=True,
                )
                # split=True halves copy+DMA latency on the kernel tail
                for c0, c1 in ([(0, 256), (256, 512)] if split else [(0, 512)]):
                    nc.scalar.copy(
                        out_sb[tt][:, 512 * hf + c0 : 512 * hf + c1],
                        ps[:, c0:c1],
                    ) if split else nc.vector.tensor_copy(
                        out_sb[tt][:, 512 * hf + c0 : 512 * hf + c1],
                        ps[:, c0:c1],
                    )
                    nc.sync.dma_start(
                        out_d[
                            128 * tt : 128 * tt + 128,
                            512 * hf + c0 : 512 * hf + c1,
                        ],
                        out_sb[tt][:, 512 * hf + c0 : 512 * hf + c1],
                    )

            # ---- pipeline ----
            # dummy matmuls during the initial DMA wait keep the PE busy for
            # the ~3.4us HAM window so real matmuls start at 2.4GHz, not 1.2
            for _ in range(36):
                wp = st_pool.tile([128, 512], F32, tag="st")
                nc.tensor.matmul(
                    wp[0:128, 0:128], warm[:], warm[:], start=True, stop=True
                )

            proj_chunk(8, 0)   # k hp0, tokens 0:256 (first xT chunk)
            proj_chunk(0, 0)   # q hp0, tokens 0:512
            proj_chunk(8, 1)   # k hp0, tokens 256:576

            _wo = wo_pool.tile([128, ND, D], BF16, tag="wo")

            # qb=3 first per head pair so tail out-projections unlock early
            units = [(hp, qb) for hp in range(8) for qb in (3, 0, 1, 2)]

            vproj(0, 3)
            vproj(0, 4)

            def P(ft, ci):
                return lambda: proj_chunk(ft, ci)

            def P2(ft, ci, half):
                return lambda: proj_part(ft, ci, half)

            def V(hf, tt):
                return lambda: vproj(hf, tt)

            def W(ft):
                return lambda: load_wcol(ft)

            def OH(tt, hf):
                return lambda: op_head(tt, hf)

            def OT(tt, hf):
                return lambda: op_tail(tt, hf)

            dma_wv1 = lambda: nc.sync.dma_start(wvs[1][:], wv_d[1])
            dma_wo = lambda: nc.sync.dma_start(_wo[:], wo_d[:])

            # (A fillers, B fillers) per slot; A lands between this slot's S
            # matmuls and the previous unit's PV, B between PV and broadcast
            # every slot gets at least one matmul filler in B — PE micro-idles
            # re-throttle the HAM clock gate to 1.2GHz for 3.4us at a time
            fills = {
                0: ([W(2), V(0, 0)], []),
                1: ([W(10), V(0, 1)], [dma_wv1]),
                2: ([V(0, 2)], [P(1, 0)]),
                3: ([P2(9, 0, 0), P2(9, 0, 1)], [P2(9, 1, 0), P2(9, 1, 1)]),
                4: ([W(3), dma_wo], [P2(10, 0, 0)]),
                5: ([W(11)], [P2(10, 0, 1)]),
                6: ([V(1, 0)], [P(2, 0)]),
                7: ([P2(10, 1, 0)], [P2(10, 1, 1)]),
                8: ([W(4), V(1, 1)], [P2(11, 0, 0)]),
                9: ([W(12)], [P2(11, 0, 1)]),
                10: ([V(1, 2)], [P(3, 0)]),
                11: ([P2(11, 1, 0)], [P2(11, 1, 1)]),
                12: ([W(5), V(1, 3)], [P2(12, 0, 0)]),
                13: ([W(13), V(1, 4)], [P2(12, 0, 1)]),
                14: ([], [P(4, 0)]),
                15: ([P2(12, 1, 0)], [P2(12, 1, 1)]),
                16: ([W(6)], [P2(13, 0, 0)]),
                17: ([W(14)], [P2(13, 0, 1)]),
                18: ([], [P(5, 0)]),
                19: ([P2(13, 1, 0)], [P2(13, 1, 1)]),
                20: ([W(7)], [P2(14, 0, 0)]),
                21: ([W(15)], [P2(14, 0, 1)]),
                22: ([], [P(6, 0)]),
                23: ([P2(14, 1, 0)], [P2(14, 1, 1)]),
                24: ([], [P2(15, 0, 0)]),
                25: ([], [P2(15, 0, 1)]),
                26: ([], [P(7, 0)]),
                27: ([P2(15, 1, 0)], [P2(15, 1, 1)]),
                28: ([], []),
                29: ([OH(3, 0)], [OT(3, 0), OH(3, 1)]),
                30: ([OT(3, 1), OH(0, 0)], [OT(0, 0), OH(0, 1)]),
                31: ([OT(0, 1), OH(1, 0)], [OT(1, 0), OH(1, 1)]),
            }

            for i, u in enumerate(units):
                if i >= 1:
                    attn_be1(units[i - 1])
                attn_fe(u)
                fa, fb = fills[i]
                for f in fa:
                    f()
                if i < 28:
                    if i >= 2:
                        attn_be2(units[i - 2])
                elif i == 28:
                    attn_be2(units[26])
                else:
                    attn_be2(units[i - 1])
                for f in fb:
                    f()
                if i == 28:
                    attn_be2(units[27])

            # tail: last unit's PV/normalize interleaved with final out proj
            attn_be1(units[31])
            op_tail(1, 1)
            op_head(2, 0)
            attn_be2(units[31])
            op_head(2, 1)
            op_tail(2, 0, split=True)
            op_tail(2, 1, split=True)

    import concourse.mybir as mybir_mod

    _split_multiwaits(nc, mybir_mod)
    return nc


def _host_inputs(x, w_qkv, b_qkv, w_out, b_out):
    scale = float(HD) ** -0.5
    w = np.asarray(w_qkv, np.float32).copy()
    b = np.asarray(b_qkv, np.float32).copy()
    w[0:D] *= scale
    b[0:D] *= scale
    w_qkvT = np.ascontiguousarray(w.T)  # [1024, 3072]
    w_qk = np.ascontiguousarray(
        w_qkvT[:, 0 : 2 * D].reshape(ND, 128, 16, 128).transpose(2, 1, 0, 3)
    ).astype(BF)  # [16 ft, 128 p, ND, 128]
    w_v = np.ascontiguousarray(
        w_qkvT[:, 2 * D :].reshape(ND, 128, 2, 512).transpose(2, 1, 0, 3)
    ).astype(BF)  # [2 hf, 128, ND, 512]
    w_o = np.ascontiguousarray(
        np.asarray(w_out, np.float32).T.reshape(ND, 128, D).transpose(1, 0, 2)
    ).astype(BF)  # [128, ND, 1024]
    b_qk = np.ascontiguousarray(b[0 : 2 * D].reshape(16, 128).T)

    # blob2 (partition 0 row): ones | b_v | b_out
    blob2 = np.zeros((1, 2176), np.float32)
    blob2[0, 0:128] = 1.0
    blob2[0, 128:1152] = b[2 * D :]
    blob2[0, 1152:2176] = np.asarray(b_out, np.float32)
    blob2 = blob2.astype(BF)

    # 0/1 band masks for S^T layout: maskT[k, q], duplicated for head pair
    kk = np.arange(128)[:, None]
    qq = np.arange(128)[None, :]
    m1 = ((kk - qq >= 0) & (kk - qq <= W2)).astype(np.float32)
    band1 = np.concatenate([m1, m1], axis=1)  # [128, 256]
    k2 = np.arange(64)[:, None] + 128
    m2 = ((k2 - qq >= 0) & (k2 - qq <= W2)).astype(np.float32)
    band2 = np.concatenate([m2, m2], axis=1)  # [64, 256]
    band2_end = np.zeros((64, 256), np.float32)

    def blob1_for(last):
        blob1 = np.zeros((128, 768), np.float32)
        blob1[:, 0:256] = band1
        blob1[0:64, 256:512] = band2
        blob1[0:64, 512:768] = band2_end if last else band2
        return blob1.astype(BF)

    blob1_mid = blob1_for(False)
    blob1_end = blob1_for(True)

    xf = np.asarray(x, np.float32).reshape(B * T, D)
    in_maps = []
    for c in range(N_CORES):
        t0 = c * TC
        bi = t0 // T
        end = min(t0 + TH, (bi + 1) * T)
        xc = np.zeros((TH, D), np.float32)
        xc[0 : end - t0] = xf[t0:end]
        in_maps.append(
            {
                "xT": np.ascontiguousarray(
                    xc.T.reshape(ND, 128, TH).transpose(1, 0, 2)
                ).astype(BF),
                "w_qk": w_qk,
                "w_v": w_v,
                "b_qk": b_qk,
                "w_o": w_o,
                "blob1": blob1_end if (end - t0) < TH else blob1_mid,
                "blob2": blob2,
            }
        )
    return in_maps


def kernel(x, w_qkv, b_qkv, w_out, b_out):
    from concourse import bass_utils

    if "nc" not in _CACHED:
        _CACHED["nc"] = _build_nc()
    nc = _CACHED["nc"]

    in_maps = _host_inputs(x, w_qkv, b_qkv, w_out, b_out)
    res = bass_utils.run_bass_kernel_spmd(
        nc, in_maps, core_ids=list(range(N_CORES))
    )
    out = np.concatenate(
        [res.results[c]["out"] for c in range(N_CORES)], axis=0
    )
    return np.ascontiguousarray(out.reshape(B, T, D)).astype(np.float32)


# revision 28
# speedup vs baseline: 1.0380x; 1.0380x over previous
"""Local sliding-window attention (B=2, T=2048, D=1024, H=16, window=128)
as a Trainium2 Bass/Tile kernel on 8 NeuronCores.

Sharding: sequence-parallel. Each core owns 512 consecutive tokens of one
batch (4 chunks x 2 batches = 8 cores) plus a 64-token halo of following
tokens (the mask lets query i attend keys [i, i+64]). No collectives.

v3 layout (v2 + startup/PE-work cuts):
  xT        [128, 8, 576] bf16 feature-major input, ONE tile filled by three
            token-range DMAs (0:256 / 256:512 / 512:576) so early projections
            start before the whole tensor lands
  v proj    token-major v_sb[tt] [128, 16, 65] bf16, col 64 = 1.0 (the
            ones column makes the PV matmul also emit softmax denominators)
  q proj    head-pair zero-padded qp_sb[hp] [128, 2, 512] bf16: plane 0 has
            the even head's features on partitions 0:64 (rest zero), plane 1
            the odd head's on 64:128. Full 128-partition contraction keeps
            every S^T matmul in one PE row-group (mixed row-groups hang).
            Bias applied on VectorE (tensor_scalar add, per-partition).
  k proj    feature-major kt_sb[hp] [128, 576] bf16, chunks 256+320
  attention per (head-pair, 128-query block): S^T = k.T q as ONE matmul per
            key tile (3D moving AP covers both head planes), exp on ScalarE
            (no additive mask), 0/1 band mask applied post-exp on VectorE,
            PV matmul emits unnormalized output + denominator row,
            1/den = exp(-ln(den)) on ScalarE, PE broadcast, normalize muls
  out proj  stationary attn tiles (feature-major) x moving w_outT, biases
            via rank-1 ones-row matmuls, per-hf output DMA

Host-side prep (numpy): transposes, bf16 casts, q-scale folding, 0/1 bands.
"""

import numpy as np
import ml_dtypes

BF = ml_dtypes.bfloat16

N_CORES = 8
B, T, D = 2, 2048, 1024
H, HD = 16, 64
W2 = 64            # window_size // 2 (look-ahead span)
TC = T // 4        # 512 own tokens per core
TH = TC + W2       # 576 with halo
NQB = TC // 128    # 4 query blocks per head
ND = D // 128      # 8 contraction tiles

_CACHED = {}


def _patch_framework(bass, mybir, tile):
    """Work around this walrus build's 1-sync-wait-per-instruction limit."""
    from concourse.vector_clock import ScopedClock

    if getattr(tile.TileContext, "_swa_patched", False):
        return

    def _drain_and_barrier(self, tick_clock, wait_clock):
        nc = self.nc
        drain_inst = nc.sync.drain()
        wait_clock.add_sem_waits(
            drain_inst.ins, ScopedClock({None: tick_clock.global_clock})
        )
        si = drain_inst.ins.sync_info
        waits = list(si.on_wait)
        if len(waits) > 1:
            si.on_wait = [waits[0]]
            for w in waits[1:]:
                extra = nc.sync.drain()
                extra.ins.sync_info = type(si)(on_wait=[w], on_update=[])
        nc.all_engine_barrier()
        assert self.sems is not None
        popped = nc._tile_sem_poison_stack.pop()
        assert popped is self._sem_poison
        nc.clear_and_free_semaphores(list(self.sems.allocated().values()))
        nc.all_engine_barrier()

    tile.TileContext._drain_and_barrier = _drain_and_barrier
    tile.TileContext._swa_patched = True


def _split_multiwaits(nc, mybir):
    """Hoist excess sync waits onto same-engine NOPs before the instruction."""
    n = 0
    for fn in nc.m.functions:
        for bb in fn.blocks:
            insts = bb.instructions
            new_list = []
            changed = False
            for inst in insts:
                si = inst.sync_info
                nw = len(si.on_wait) if si is not None and si.on_wait else 0
                if nw > 1:
                    waits = list(si.on_wait)
                    for j, w in enumerate(waits[:-1]):
                        nop = mybir.InstNoOp(
                            name=f"{inst.name}-wsplit{j}", ins=[], outs=[]
                        )
                        nop.engine = inst.engine
                        nop.sync_info = mybir.SyncInfo(on_wait=[w], on_update=[])
                        new_list.append(nop)
                        n += 1
                    si.on_wait = waits[-1:]
                    changed = True
                new_list.append(inst)
            if changed:
                insts.clear()
                insts.extend(new_list)
    return n


def _build_nc():
    import concourse.bass as bass
    import concourse.mybir as mybir
    import concourse.tile as tile

    _patch_framework(bass, mybir, tile)

    F32 = mybir.dt.float32
    BF16 = mybir.dt.bfloat16
    AF = mybir.ActivationFunctionType

    nc = bass.Bass("TRN2")

    xT_d = nc.dram_tensor("xT", [128, ND, TH], BF16, kind="ExternalInput")
    wqk_d = nc.dram_tensor("w_qk", [16, 128, ND, 128], BF16, kind="ExternalInput")
    wv_d = nc.dram_tensor("w_v", [2, 128, ND, 512], BF16, kind="ExternalInput")
    bqk_d = nc.dram_tensor("b_qk", [128, 16], F32, kind="ExternalInput")
    wo_d = nc.dram_tensor("w_o", [128, ND, D], BF16, kind="ExternalInput")
    # blob1: band1 [128,256] at cols 0:256; band2 [64,2,256] rows 0:64 cols 256:768
    blob1_d = nc.dram_tensor("blob1", [128, 768], BF16, kind="ExternalInput")
    # blob2 (partition 0): ones 0:128 | b_v 128:1152 | b_out 1152:2176
    blob2_d = nc.dram_tensor("blob2", [1, 2176], BF16, kind="ExternalInput")
    out_d = nc.dram_tensor("out", [TC, D], F32, kind="ExternalOutput")

    with tile.TileContext(nc) as tc:
        with (
            tc.tile_pool(name="persist", bufs=1) as persist,
            tc.tile_pool(name="consts", bufs=1) as consts,
            tc.tile_pool(name="wcol", bufs=4) as wcol_pool,
            tc.tile_pool(name="wv", bufs=2) as wv_pool,
            tc.tile_pool(name="wo", bufs=1) as wo_pool,
            tc.tile_pool(name="psmm", bufs=3, space="PSUM") as mm_pool,
            tc.tile_pool(name="psst", bufs=2, space="PSUM") as st_pool,
            tc.tile_pool(name="psop", bufs=3, space="PSUM") as op_pool,
            tc.tile_pool(name="pp", bufs=3) as p_pool,
            tc.tile_pool(name="rdp", bufs=3) as rd_pool,
        ):
            # ---- persistent SBUF ----
            xT = persist.tile([128, ND, TH], BF16, tag="xT", name="xT")
            # split by token range so early consumers start sooner
            nc.sync.dma_start(xT[:, :, 0:256], xT_d[:, :, 0:256])

            _wcols = {}

            def load_wcol(ft):
                wcol = wcol_pool.tile([128, ND, 128], BF16, tag="wcol")
                nc.sync.dma_start(wcol[:], wqk_d[ft])
                _wcols[ft] = wcol

            load_wcol(8)
            load_wcol(0)

            bqk = consts.tile([128, 16], F32, tag="bqk")
            nc.sync.dma_start(bqk[:], bqk_d[:])
            nc.sync.dma_start(xT[:, :, 256:512], xT_d[:, :, 256:512])
            nc.sync.dma_start(xT[:, :, 512:576], xT_d[:, :, 512:576])

            wvs = [
                wv_pool.tile([128, ND, 512], BF16, tag="wv", name=f"wv{i}")
                for i in range(2)
            ]
            nc.sync.dma_start(wvs[0][:], wv_d[0])  # wv1 DMA'd later

            blob2 = consts.tile([1, 2176], BF16, tag="blob2")
            nc.sync.dma_start(blob2[:], blob2_d[:])
            blob1 = consts.tile([128, 768], BF16, tag="blob1")
            nc.sync.dma_start(blob1[:], blob1_d[:])
            load_wcol(9)
            load_wcol(1)

            ones = blob2[0:1, 0:128]
            band1 = blob1[:, 0:256]

            def band2(mi):
                return blob1[0:64, 256 + 256 * mi : 512 + 256 * mi]

            def bv(hf):
                return blob2[0:1, 128 + 512 * hf : 640 + 512 * hf]

            def bo(hf):
                return blob2[0:1, 1152 + 512 * hf : 1664 + 512 * hf]

            qp_sb = [
                persist.tile([128, 2, TC], BF16, tag=f"qp{hp}", name=f"qp{hp}")
                for hp in range(8)
            ]
            kt_sb = [
                persist.tile([128, TH], BF16, tag=f"kt{hp}", name=f"kt{hp}")
                for hp in range(8)
            ]
            v_sb = [
                persist.tile([128, H, HD + 1], BF16, tag=f"v{tt}", name=f"v{tt}")
                for tt in range(5)
            ]
            attn_sb = [
                persist.tile([128, TC], BF16, tag=f"at{pt}", name=f"at{pt}")
                for pt in range(8)
            ]
            out_sb = [
                persist.tile([128, D], F32, tag=f"o{tt}", name=f"o{tt}")
                for tt in range(4)
            ]

            # one-time zero/one fills on the (otherwise idle) GpSimd engine.
            # warm tile first: the HAM warmup matmuls depend on it and must
            # run during the initial DMA wait.
            warm = persist.tile([128, 128], BF16, tag="warm", name="warm")
            nc.gpsimd.memset(warm[0:1, 0:1], 0.0)
            # v ones columns next (needed by the first PV matmuls): they make
            # PV emit softmax denominators in row HD.
            for tt in range(5):
                nc.gpsimd.memset(v_sb[tt][:, :, HD : HD + 1], 1.0)
            for hp in range(8):
                nc.gpsimd.memset(qp_sb[hp][64:128, 0, :], 0.0)
                nc.gpsimd.memset(qp_sb[hp][0:64, 1, :], 0.0)

            # ---- q/k projection, emittable in dt-halves for finer filling ----
            _pcs = {}

            def proj_part(ft, ci, half):
                # q (ft<8): one 512-token chunk (ci==0 only); k: 256+320
                if ft < 8:
                    c0, w = 0, 512
                else:
                    c0, w = (0, 256) if ci == 0 else (256, 320)
                wcol = _wcols[ft]
                if half == 0:
                    ps = mm_pool.tile([128, 512], F32, tag="psmm")
                    _pcs[(ft, ci)] = ps
                    dts = range(0, 4)
                else:
                    ps = _pcs.pop((ft, ci))
                    dts = range(4, ND)
                for dt in dts:
                    nc.tensor.matmul(
                        ps[:, 0:w],
                        wcol[:, dt, :],
                        xT[:, dt, c0 : c0 + w],
                        start=(dt == 0),
                        stop=(dt == ND - 1),
                    )
                if half == 0:
                    return
                if ft < 8:
                    nc.vector.tensor_scalar_add(
                        qp_sb[ft][0:64, 0, c0 : c0 + w],
                        ps[0:64, 0:w],
                        bqk[0:64, ft : ft + 1],
                    )
                    nc.vector.tensor_scalar_add(
                        qp_sb[ft][64:128, 1, c0 : c0 + w],
                        ps[64:128, 0:w],
                        bqk[64:128, ft : ft + 1],
                    )
                else:
                    nc.vector.tensor_scalar_add(
                        kt_sb[ft - 8][:, c0 : c0 + w],
                        ps[:, 0:w],
                        bqk[:, ft : ft + 1],
                    )

            def proj_chunk(ft, ci):
                proj_part(ft, ci, 0)
                proj_part(ft, ci, 1)

            # ---- v projection for one (feature-half, token-tile) ----
            def vproj(hf, tt):
                tsz = 128 if tt < 4 else 64
                ps = mm_pool.tile([128, 512], F32, tag="psmm")
                for dt in range(ND):
                    nc.tensor.matmul(
                        ps[0:tsz, :],
                        xT[:, dt, 128 * tt : 128 * tt + tsz],
                        wvs[hf][:, dt, :],
                        start=(dt == 0),
                        stop=False,
                    )
                nc.tensor.matmul(
                    ps[0:tsz, :],
                    blob2[0:1, 0:tsz],
                    bv(hf),
                    start=False,
                    stop=True,
                )
                nc.scalar.copy(
                    v_sb[tt][0:tsz, 8 * hf : 8 * hf + 8, 0:HD],
                    ps[0:tsz, :].rearrange("p (h f) -> p h f", h=8),
                )

            # ---- attention, software-pipelined in three stages ----
            # fe: S matmuls + exp + band mask   (unit state in _ust)
            # be1 (next slot): PV matmuls + ln/exp reciprocal
            # be2 (slot after): PE broadcast + normalize muls
            _ust = {}

            def attn_fe(u):
                hp, qb = u
                q0 = 128 * qb
                mi = 1 if qb == NQB - 1 else 0
                q2 = qp_sb[hp][:, 0:2, q0 : q0 + 128]  # both head planes
                kt = kt_sb[hp]
                st = st_pool.tile([128, 512], F32, tag="st")
                # S^T tiles: keys on partitions, (plane, query) on free axis.
                nc.tensor.matmul(
                    st[0:128, 0:256], kt[:, q0 : q0 + 128], q2,
                    start=True, stop=True,
                )
                nc.tensor.matmul(
                    st[0:64, 256:512], kt[:, q0 + 128 : q0 + 192], q2,
                    start=True, stop=True,
                )
                par = p_pool.tile([128, 256], BF16, tag="par")
                pbr = p_pool.tile([64, 256], BF16, tag="pbr")
                nc.scalar.activation(par[:], st[0:128, 0:256], AF.Exp)
                nc.scalar.activation(pbr[:], st[0:64, 256:512], AF.Exp)
                # 0/1 band masks applied post-exp (cheaper than mask matmuls)
                pa = p_pool.tile([128, 256], BF16, tag="pa")
                pb = p_pool.tile([64, 256], BF16, tag="pb")
                nc.vector.tensor_mul(pa[:], par[:], band1)
                nc.vector.tensor_mul(pb[:], pbr[:], band2(mi))
                _ust[u] = [pa, pb]

            def attn_be1(u):
                hp, qb = u
                pa, pb = _ust[u]
                op = op_pool.tile([128, 512], F32, tag="op")
                vE1 = v_sb[qb][:, 2 * hp, :]
                vO1 = v_sb[qb][:, 2 * hp + 1, :]
                vE2 = v_sb[qb + 1][0:64, 2 * hp, :]
                vO2 = v_sb[qb + 1][0:64, 2 * hp + 1, :]
                # pa-consumers first (pb's exp+mask lands later); one
                # accumulation group: start=True clears the WHOLE bank's
                # has_written bits, so only the first matmul may set it
                nc.tensor.matmul(
                    op[0 : HD + 1, 0:128], vE1, pa[:, 0:128],
                    start=True, stop=False,
                )
                nc.tensor.matmul(
                    op[0 : HD + 1, 128:256], vO1, pa[:, 128:256],
                    start=False, stop=False, skip_group_check=True,
                )
                nc.tensor.matmul(
                    op[0 : HD + 1, 0:128], vE2, pb[:, 0:128],
                    start=False, stop=False, skip_group_check=True,
                )
                nc.tensor.matmul(
                    op[0 : HD + 1, 128:256], vO2, pb[:, 128:256],
                    start=False, stop=True, skip_group_check=True,
                )
                # 1/den = exp(-ln(den)) on ScalarE
                rd = rd_pool.tile([1, 256], F32, tag="rd")
                nc.scalar.activation(rd[:], op[HD : HD + 1, 0:256], AF.Ln)
                rdb = rd_pool.tile([1, 256], BF16, tag="rdb")
                nc.scalar.activation(rdb[:], rd[:], AF.Exp, scale=-1.0)
                _ust[u] = [op, rdb]

            def attn_be2(u):
                hp, qb = u
                q0 = 128 * qb
                op, rdb = _ust.pop(u)
                nc.tensor.matmul(
                    op[0:64, 256:512],
                    blob2[0:1, 0:64],
                    rdb[:],
                    start=True, stop=True,
                )
                bc = p_pool.tile([64, 256], F32, tag="bc")
                nc.vector.tensor_copy(bc[:], op[0:64, 256:512])
                nc.vector.tensor_mul(
                    attn_sb[hp][0:64, q0 : q0 + 128],
                    op[0:HD, 0:128],
                    bc[:, 0:128],
                )
                nc.vector.tensor_mul(
                    attn_sb[hp][64:128, q0 : q0 + 128],
                    op[0:HD, 128:256],
                    bc[:, 128:256],
                )

            # ---- out projection, split so dt 0..6 (which only need head
            # pairs 0..6) can run as filler before the last attentions ----
            _ops = {}

            def op_head(tt, hf):
                ps = mm_pool.tile([128, 512], F32, tag="psmm")
                _ops[(tt, hf)] = ps
                for dt in range(7):
                    nc.tensor.matmul(
                        ps[:],
                        attn_sb[dt][:, 128 * tt : 128 * tt + 128],
                        _wo[:, dt, 512 * hf : 512 * hf + 512],
                        start=(dt == 0),
                        stop=False,
                    )

            def op_tail(tt, hf, split=False):
                ps = _ops.pop((tt, hf))
                nc.tensor.matmul(
                    ps[:],
                    attn_sb[7][:, 128 * tt : 128 * tt + 128],
                    _wo[:, 7, 512 * hf : 512 * hf + 512],
                    start=False,
                    stop=False,
                )
                nc.tensor.matmul(
                    ps[:], blob2[0:1, 0:128], bo(hf), start=False, stop=True,
                )
                # split=True halves copy+DMA latency on the kernel tail
                for c0, c1 in ([(0, 256), (256, 512)] if split else [(0, 512)]):
                    nc.scalar.copy(
                        out_sb[tt][:, 512 * hf + c0 : 512 * hf + c1],
                        ps[:, c0:c1],
                    )
                    nc.sync.dma_start(
                        out_d[
                            128 * tt : 128 * tt + 128,
                            512 * hf + c0 : 512 * hf + c1,
                        ],
                        out_sb[tt][:, 512 * hf + c0 : 512 * hf + c1],
                    )

            # ---- pipeline ----
            # dummy matmuls during the initial DMA wait keep the PE busy for
            # the ~3.4us HAM window so real matmuls start at 2.4GHz, not 1.2
            for _ in range(36):
                wp = st_pool.tile([128, 512], F32, tag="st")
                nc.tensor.matmul(
                    wp[0:128, 0:128], warm[:], warm[:], start=True, stop=True
                )

            proj_chunk(8, 0)   # k hp0, tokens 0:256 (first xT chunk)
            proj_chunk(0, 0)   # q hp0, tokens 0:512
            proj_chunk(8, 1)   # k hp0, tokens 256:576

            _wo = wo_pool.tile([128, ND, D], BF16, tag="wo")

            # qb=3 first per head pair so tail out-projections unlock early
            units = [(hp, qb) for hp in range(8) for qb in (3, 0, 1, 2)]

            vproj(0, 3)
            vproj(0, 4)

            def P(ft, ci):
                return lambda: proj_chunk(ft, ci)

            def P2(ft, ci, half):
                return lambda: proj_part(ft, ci, half)

            def V(hf, tt):
                return lambda: vproj(hf, tt)

            def W(ft):
                return lambda: load_wcol(ft)

            def OH(tt, hf):
                return lambda: op_head(tt, hf)

            def OT(tt, hf):
                return lambda: op_tail(tt, hf)

            dma_wv1 = lambda: nc.sync.dma_start(wvs[1][:], wv_d[1])
            dma_wo = lambda: nc.sync.dma_start(_wo[:], wo_d[:])

            # (A fillers, B fillers) per slot; A lands between this slot's S
            # matmuls and the previous unit's PV, B between PV and broadcast
            # every slot gets at least one matmul filler in B — PE micro-idles
            # re-throttle the HAM clock gate to 1.2GHz for 3.4us at a time
            fills = {
                0: ([W(2), V(0, 0)], []),
                1: ([W(10), V(0, 1)], [dma_wv1]),
                2: ([V(0, 2)], [P(1, 0)]),
                3: ([P2(9, 0, 0), P2(9, 0, 1)], [P2(9, 1, 0), P2(9, 1, 1)]),
                4: ([W(3), dma_wo], [P2(10, 0, 0)]),
                5: ([W(11)], [P2(10, 0, 1)]),
                6: ([V(1, 0)], [P(2, 0)]),
                7: ([P2(10, 1, 0)], [P2(10, 1, 1)]),
                8: ([W(4), V(1, 1)], [P2(11, 0, 0)]),
                9: ([W(12)], [P2(11, 0, 1)]),
                10: ([V(1, 2)], [P(3, 0)]),
                11: ([P2(11, 1, 0)], [P2(11, 1, 1)]),
                12: ([W(5), V(1, 3)], [P2(12, 0, 0)]),
                13: ([W(13), V(1, 4)], [P2(12, 0, 1)]),
                14: ([], [P(4, 0)]),
                15: ([P2(12, 1, 0)], [P2(12, 1, 1)]),
                16: ([W(6)], [P2(13, 0, 0)]),
                17: ([W(14)], [P2(13, 0, 1)]),
                18: ([], [P(5, 0)]),
                19: ([P2(13, 1, 0)], [P2(13, 1, 1)]),
                20: ([W(7)], [P2(14, 0, 0)]),
                21: ([W(15)], [P2(14, 0, 1)]),
                22: ([], [P(6, 0)]),
                23: ([P2(14, 1, 0)], [P2(14, 1, 1)]),
                24: ([], [P2(15, 0, 0)]),
                25: ([], [P2(15, 0, 1)]),
                26: ([], [P(7, 0)]),
                27: ([P2(15, 1, 0)], [P2(15, 1, 1)]),
                28: ([], []),
                29: ([OH(3, 0)], [OT(3, 0), OH(3, 1)]),
                30: ([OT(3, 1), OH(0, 0)], [OT(0, 0), OH(0, 1)]),
                31: ([OT(0, 1), OH(1, 0)], [OT(1, 0), OH(1, 1)]),
            }

            for i, u in enumerate(units):
                if i >= 1:
                    attn_be1(units[i - 1])
                attn_fe(u)
                fa, fb = fills[i]
                for f in fa:
                    f()
                if i < 28:
                    if i >= 2:
                        attn_be2(units[i - 2])
                elif i == 28:
                    attn_be2(units[26])
                else:
                    attn_be2(units[i - 1])
                for f in fb:
                    f()
                if i == 28:
                    attn_be2(units[27])

            # tail: last unit's PV/normalize interleaved with final out proj
            attn_be1(units[31])
            op_tail(1, 1)
            op_head(2, 0)
            attn_be2(units[31])
            op_head(2, 1)
            op_tail(2, 0, split=True)
            op_tail(2, 1, split=True)

    import concourse.mybir as mybir_mod

    _split_multiwaits(nc, mybir_mod)
    return nc


def _host_inputs(x, w_qkv, b_qkv, w_out, b_out):
    scale = float(HD) ** -0.5
    w = np.asarray(w_qkv, np.float32).copy()
    b = np.asarray(b_qkv, np.float32).copy()
    w[0:D] *= scale
    b[0:D] *= scale
    w_qkvT = np.ascontiguousarray(w.T)  # [1024, 3072]
    w_qk = np.ascontiguousarray(
        w_qkvT[:, 0 : 2 * D].reshape(ND, 128, 16, 128).transpose(2, 1, 0, 3)
    ).astype(BF)  # [16 ft, 128 p, ND, 128]
    w_v = np.ascontiguousarray(
        w_qkvT[:, 2 * D :].reshape(ND, 128, 2, 512).transpose(2, 1, 0, 3)
    ).astype(BF)  # [2 hf, 128, ND, 512]
    w_o = np.ascontiguousarray(
        np.asarray(w_out, np.float32).T.reshape(ND, 128, D).transpose(1, 0, 2)
    ).astype(BF)  # [128, ND, 1024]
    b_qk = np.ascontiguousarray(b[0 : 2 * D].reshape(16, 128).T)

    # blob2 (partition 0 row): ones | b_v | b_out
    blob2 = np.zeros((1, 2176), np.float32)
    blob2[0, 0:128] = 1.0
    blob2[0, 128:1152] = b[2 * D :]
    blob2[0, 1152:2176] = np.asarray(b_out, np.float32)
    blob2 = blob2.astype(BF)

    # 0/1 band masks for S^T layout: maskT[k, q], duplicated for head pair
    kk = np.arange(128)[:, None]
    qq = np.arange(128)[None, :]
    m1 = ((kk - qq >= 0) & (kk - qq <= W2)).astype(np.float32)
    band1 = np.concatenate([m1, m1], axis=1)  # [128, 256]
    k2 = np.arange(64)[:, None] + 128
    m2 = ((k2 - qq >= 0) & (k2 - qq <= W2)).astype(np.float32)
    band2 = np.concatenate([m2, m2], axis=1)  # [64, 256]
    band2_end = np.zeros((64, 256), np.float32)

    def blob1_for(last):
        blob1 = np.zeros((128, 768), np.float32)
        blob1[:, 0:256] = band1
        blob1[0:64, 256:512] = band2
        blob1[0:64, 512:768] = band2_end if last else band2
        return blob1.astype(BF)

    blob1_mid = blob1_for(False)
    blob1_end = blob1_for(True)

    xf = np.asarray(x, np.float32).reshape(B * T, D)
    in_maps = []
    for c in range(N_CORES):
        t0 = c * TC
        bi = t0 // T
        end = min(t0 + TH, (bi + 1) * T)
        xc = np.zeros((TH, D), np.float32)
        xc[0 : end - t0] = xf[t0:end]
        in_maps.append(
            {
                "xT": np.ascontiguousarray(
                    xc.T.reshape(ND, 128, TH).transpose(1, 0, 2)
                ).astype(BF),
                "w_qk": w_qk,
                "w_v": w_v,
                "b_qk": b_qk,
                "w_o": w_o,
                "blob1": blob1_end if (end - t0) < TH else blob1_mid,
                "blob2": blob2,
            }
        )
    return in_maps


def kernel(x, w_qkv, b_qkv, w_out, b_out):
    from concourse import bass_utils

    if "nc" not in _CACHED:
        _CACHED["nc"] = _build_nc()
    nc = _CACHED["nc"]

    in_maps = _host_inputs(x, w_qkv, b_qkv, w_out, b_out)
    res = bass_utils.run_bass_kernel_spmd(
        nc, in_maps, core_ids=list(range(N_CORES))
    )
    out = np.concatenate(
        [res.results[c]["out"] for c in range(N_CORES)], axis=0
    )
    return np.ascontiguousarray(out.reshape(B, T, D)).astype(np.float32)
